# revision 1
# baseline (speedup 1.0000x reference)
"""Trainium2 Bass kernel for nn_AttentionCompiled (dense transformer attention).

B=8, N=1024, C=768, H=12 heads, D=64. Per-head LayerNorm on q/k, softmax
attention, output projection.

Strategy: pure data parallelism — one batch element per NeuronCore (B=8 ==
n_cores), weights replicated, no collectives.

Math folding (validated vs reference to ~1e-6):
 - LayerNorm centering is folded into the QKV weights: Wq_h <- (I - J/64) Wq_h
   (and same for k). Row-sums of centered vectors vanish, so
   (q-mu_q).(k-mu_k) == qc.kc with both pre-centered by the weights.
 - 1/sigma_q (and the 1/sqrt(D) attention scale) is folded into q before the
   QK matmul: aq[n] = 1/sqrt(sum_d qc^2 + 64*eps) == (1/8)/sqrt(var+eps).
 - 1/sigma_k is applied as the per-partition `scale` operand of the Exp
   activation on S^T tiles (partition axis = m = key index).
 - Softmax has no max-subtraction: |scores| <= 8 by Cauchy-Schwarz on the
   normalized vectors, so exp is always in-range in fp32.
 - Softmax denominators come free from an appended ones-column on V:
   out^T = [v | 1]^T @ P^T gives rows 0..63 = unnormalized output, row 64 =
   sum_m P^T[m, n].
 - All activations live in transposed ([feature, token]) layouts so the
   attention chain needs zero on-chip transposes; x is pre-transposed on the
   CPU (free), and the final projection (lhsT = O^T slice) lands the output
   back in natural [token, feature] layout.
"""

import sys
import numpy as np
from contextlib import ExitStack

if "/opt/trn_rl_repo" not in sys.path:
    sys.path.insert(0, "/opt/trn_rl_repo")

import concourse.bass as bass
import concourse.bacc as bacc
import concourse.tile as tile
from concourse import mybir

F32 = mybir.dt.float32
F32R = mybir.dt.float32r

N = 1024     # tokens per core
C = 768      # channels
H = 12       # heads
D = 64       # head dim
NT = N // 128   # 8 token tiles
CT = C // 128   # 6 channel tiles
NB = N // 512   # 2 free-dim blocks of 512
EPS = 1e-5

USE_F32R = True  # fp32 matmuls run 4 cyc/row; float32r runs 1 cyc/row at N>=256


def build(use_f32r: bool = USE_F32R, compile_module: bool = True,
          squares_on_gpsimd: bool = False) -> bass.Bass:
    nc = bacc.Bacc()

    xT = nc.declare_dram_parameter("xT", [C, N], F32, isOutput=False)         # x[b].T
    wq = nc.declare_dram_parameter("wqcT", [C, C], F32, isOutput=False)       # centered Wq^T  [c, hd]
    wk = nc.declare_dram_parameter("wkcT", [C, C], F32, isOutput=False)       # centered Wk^T
    wv = nc.declare_dram_parameter("wvT", [C, C], F32, isOutput=False)        # Wv^T
    wp = nc.declare_dram_parameter("wpT", [C, C], F32, isOutput=False)        # proj_weight^T
    seg = nc.declare_dram_parameter("seg", [128, CT, H], F32, isOutput=False)  # per-c-tile head-segment-sum lhsT
    ind = nc.declare_dram_parameter("ind", [H, C], F32, isOutput=False)       # head->channel block indicator (bcast lhsT)
    id12 = nc.declare_dram_parameter("id12", [H, H], F32, isOutput=False)     # identity for PE transpose
    out_ext = nc.declare_dram_parameter("out", [N, C], F32, isOutput=True)

    MMD = F32R if use_f32r else F32

    def mm(ap):
        # bitcast for APs out of fp32-typed tensors; no-op if already f32r
        if use_f32r and ap.dtype != F32R:
            return ap.bitcast(F32R)
        return ap

    with tile.TileContext(nc) as tc, ExitStack() as ctx:
        # ---- pools ----
        # persistent single-buffer tensors get their own tags in `persist`
        persist = ctx.enter_context(tc.tile_pool(name="persist", bufs=1))
        # xT tiles and O^T tiles share slots (disjoint lifetimes), same for wv/wp
        xo_pool = ctx.enter_context(tc.tile_pool(name="xo", bufs=6))
        vp_pool = ctx.enter_context(tc.tile_pool(name="vp", bufs=6))
        work = ctx.enter_context(tc.tile_pool(name="work", bufs=2))
        ptp = ctx.enter_context(tc.tile_pool(name="ptp", bufs=4))
        epi = ctx.enter_context(tc.tile_pool(name="epi", bufs=2))
        rows = ctx.enter_context(tc.tile_pool(name="rows", bufs=1))
        dramp = ctx.enter_context(tc.tile_pool(name="dramp", bufs=2, space="DRAM"))
        psA = ctx.enter_context(tc.tile_pool(name="psA", bufs=2, space="PSUM"))
        psB = ctx.enter_context(tc.tile_pool(name="psB", bufs=4, space="PSUM"))

        # ---- load inputs to SBUF ----
        xt_sb = []
        for r in range(CT):
            t = xo_pool.tile([128, N], MMD, name=f"xt{r}", tag="xo")
            nc.sync.dma_start(out=t, in_=mm(xT[128 * r:128 * (r + 1), :]))
            xt_sb.append(t)

        def load_w(dram, nm, pool, tag, per_tile_tags=False):
            tiles = []
            for r in range(CT):
                t = pool.tile(
                    [128, C], MMD, name=f"{nm}{r}",
                    tag=(f"{tag}{r}" if per_tile_tags else tag),
                )
                nc.sync.dma_start(out=t, in_=mm(dram[128 * r:128 * (r + 1), :]))
                tiles.append(t)
            return tiles

        wq_sb = load_w(wq, "wq", persist, "wq", per_tile_tags=True)
        wk_sb = load_w(wk, "wk", persist, "wk", per_tile_tags=True)
        wv_sb = load_w(wv, "wv", vp_pool, "vp")

        seg_sb = rows.tile([128, CT, H], MMD, name="seg", tag="seg")
        nc.sync.dma_start(out=seg_sb, in_=mm(seg[:, :, :]))
        ind_sb = rows.tile([H, C], MMD, name="ind", tag="ind")
        nc.sync.dma_start(out=ind_sb, in_=mm(ind[:, :]))
        id12_sb = rows.tile([H, H], F32, name="id12", tag="id12")
        nc.sync.dma_start(out=id12_sb, in_=id12[:, :])

        epsq_sb = rows.tile([H, 1], F32, name="epsq", tag="epsq")
        nc.vector.memset(epsq_sb, float(D) * EPS)
        epsk_sb = rows.tile([H, 1], F32, name="epsk", tag="epsk")
        nc.vector.memset(epsk_sb, EPS)

        # ---- stage A: q^c.T and k^c.T  ([hd, n] layouts, 6 c-tiles each) ----
        qh_sb, kh_sb = [], []
        for nm, wt, outl in (("qh", wq_sb, qh_sb), ("kh", wk_sb, kh_sb)):
            for r in range(CT):
                ps = psA.tile([128, N], F32, name=f"ps_{nm}{r}", tag="a")
                for kc in range(CT):
                    for nb in range(NB):
                        nc.tensor.matmul(
                            ps[:, 512 * nb:512 * (nb + 1)],
                            lhsT=mm(wt[kc][:, 128 * r:128 * (r + 1)]),
                            rhs=mm(xt_sb[kc][:, 512 * nb:512 * (nb + 1)]),
                            start=(kc == 0),
                            stop=(kc == CT - 1),
                        )
                t = persist.tile([128, N], MMD, name=f"{nm}sb{r}", tag=f"{nm}{r}")
                nc.vector.tensor_copy(out=t, in_=ps)
                outl.append(t)

        # ---- stage A2: V in natural layout with ones column: [m, h, 65] ----
        v_sb = []
        for mt in range(NT):
            ps = psA.tile([128, C], F32, name=f"ps_v{mt}", tag="a")
            for kc in range(CT):
                for vo, vn in ((0, 512), (512, 256)):
                    nc.tensor.matmul(
                        ps[:, vo:vo + vn],
                        lhsT=mm(xt_sb[kc][:, 128 * mt:128 * (mt + 1)]),
                        rhs=mm(wv_sb[kc][:, vo:vo + vn]),
                        start=(kc == 0),
                        stop=(kc == CT - 1),
                    )
            t = persist.tile([128, H, D + 1], MMD, name=f"vsb{mt}", tag=f"v{mt}")
            # ones column for the softmax-denominator trick: fill with 1.0
            # first (contiguous memset), then overwrite cols 0..D-1 with v
            nc.vector.memset(t.bitcast(F32) if use_f32r else t, 1.0)
            nc.vector.tensor_copy(
                out=t[:, :, 0:D], in_=ps.rearrange("p (h d) -> p h d", h=H)
            )
            v_sb.append(t)

        # ---- stage B: per-head inverse std rows  aq, ak [12, 1024] ----
        stat_sb = {}
        for nm, src, eps_t, in (("aq", qh_sb, epsq_sb), ("ak", kh_sb, epsk_sb)):
            ps = psA.tile([H, N], F32, name=f"ps_{nm}", tag="a")
            for r in range(CT):
                sq = work.tile([128, N], MMD, name=f"sq_{nm}{r}", tag="sq")
                sq_eng = nc.gpsimd if squares_on_gpsimd else nc.vector
                sq_eng.tensor_mul(out=sq, in0=src[r], in1=src[r])
                for nb in range(NB):
                    nc.tensor.matmul(
                        ps[:, 512 * nb:512 * (nb + 1)],
                        lhsT=mm(seg_sb[:, r, :]),
                        rhs=mm(sq[:, 512 * nb:512 * (nb + 1)]),
                        start=(r == 0),
                        stop=(r == CT - 1),
                    )
            t = persist.tile([H, N], F32, name=f"{nm}sb", tag=nm)
            # sigma-like row: sqrt(scale*sumsq + eps); then reciprocal
            nc.scalar.activation(
                out=t, in_=ps, func=mybir.ActivationFunctionType.Sqrt,
                bias=eps_t, scale=(1.0 if nm == "aq" else 1.0 / D),
            )
            nc.vector.reciprocal(out=t, in_=t)
            stat_sb[nm] = t
        aq_sb, ak_sb = stat_sb["aq"], stat_sb["ak"]
        if use_f32r:
            aq_r = persist.tile([H, N], F32R, name="aq_r", tag="aq_r")
            nc.vector.tensor_copy(out=aq_r, in_=aq_sb)
            aq_sb = aq_r

        # ---- stage C: scale q by broadcast(aq) (folds LN sigma + 1/sqrt(D)) ----
        for r in range(CT):
            ps = psA.tile([128, N], F32, name=f"ps_bq{r}", tag="a")
            for nb in range(NB):
                nc.tensor.matmul(
                    ps[:, 512 * nb:512 * (nb + 1)],
                    lhsT=mm(ind_sb[:, 128 * r:128 * (r + 1)]),
                    rhs=mm(aq_sb[:, 512 * nb:512 * (nb + 1)]),
                    start=True, stop=True,
                )
            nc.vector.tensor_mul(out=qh_sb[r], in0=qh_sb[r], in1=ps)

        # ---- stage D: ak columns per m-tile via PE transpose: akT[mt] [128, 12] ----
        akT_sb = []
        for mt in range(NT):
            ps = psB.tile([128, H], F32, name=f"ps_akT{mt}", tag="s")
            nc.tensor.transpose(
                out=ps, in_=ak_sb[:, 128 * mt:128 * (mt + 1)], identity=id12_sb
            )
            t = persist.tile([128, H], F32, name=f"akT{mt}", tag=f"akT{mt}")
            nc.vector.tensor_copy(out=t, in_=ps)
            akT_sb.append(t)

        # ---- stage E: attention, head pairs (row-tiled K=64 matmuls) ----
        ot_sb = []
        for r in range(CT):
            t = xo_pool.tile([128, N], MMD, name=f"ot{r}", tag="xo")
            ot_sb.append(t)

        for pr in range(CT):  # head pair = c-tile of qh/kh
            ot_ps = [
                psA.tile([D + 1, N], F32, name=f"ps_ot{pr}_{j}", tag="a")
                for j in range(2)
            ]
            for mt in range(NT):
                for nb in range(NB):
                    for j in range(2):
                        h = 2 * pr + j
                        s_ps = psB.tile([128, 512], F32, name=f"ps_s{h}_{mt}_{nb}", tag="s")
                        nc.tensor.matmul(
                            s_ps,
                            lhsT=mm(kh_sb[pr][64 * j:64 * (j + 1), 128 * mt:128 * (mt + 1)]),
                            rhs=mm(qh_sb[pr][64 * j:64 * (j + 1), 512 * nb:512 * (nb + 1)]),
                            start=True, stop=True,
                        )
                        pt = ptp.tile([128, 512], MMD, name=f"pt{h}_{mt}_{nb}", tag="pt")
                        nc.scalar.activation(
                            out=pt, in_=s_ps, func=mybir.ActivationFunctionType.Exp,
                            scale=akT_sb[mt][:, h:h + 1],
                        )
                        nc.tensor.matmul(
                            ot_ps[j][:, 512 * nb:512 * (nb + 1)],
                            lhsT=mm(v_sb[mt][:, h, :]),
                            rhs=mm(pt),
                            start=(mt == 0),
                            stop=(mt == NT - 1),
                        )
            # epilogue: divide by softmax denominator (row 64 of ot_ps)
            for j in range(2):
                h = 2 * pr + j
                tmp = epi.tile([D + 1, N], F32, name=f"tmp{h}", tag="tmp")
                rbc = epi.tile([D, N], F32, name=f"rbc{h}", tag="rbc")
                nc.vector.reciprocal(out=tmp[D:D + 1, :], in_=ot_ps[j][D:D + 1, :])
                dn_dram = dramp.tile([1, N], F32, name=f"dnd{h}", tag="dnd")
                nc.gpsimd.dma_start(out=dn_dram, in_=tmp[D:D + 1, :])
                bcast_ap = bass.AP(
                    tensor=dn_dram.tensor, offset=dn_dram.offset,
                    ap=[[0, D]] + [list(a) for a in dn_dram.ap[1:]],
                )
                nc.gpsimd.dma_start(out=rbc, in_=bcast_ap)
                if j == 0:
                    nc.vector.tensor_mul(
                        out=ot_sb[pr][0:D, :], in0=ot_ps[j][0:D, :], in1=rbc
                    )
                else:
                    nc.vector.tensor_mul(out=tmp[0:D, :], in0=ot_ps[j][0:D, :], in1=rbc)
                    nc.gpsimd.dma_start(out=ot_sb[pr][D:2 * D, :], in_=mm(tmp[0:D, :]))

        # ---- stage F: projection (lands output in natural [n, c] layout) ----
        wp_sb = load_w(wp, "wp", vp_pool, "vp")
        for nt in range(NT):
            ps = psA.tile([128, C], F32, name=f"ps_pj{nt}", tag="a")
            for kc in range(CT):
                for vo, vn in ((0, 512), (512, 256)):
                    nc.tensor.matmul(
                        ps[:, vo:vo + vn],
                        lhsT=mm(ot_sb[kc][:, 128 * nt:128 * (nt + 1)]),
                        rhs=mm(wp_sb[kc][:, vo:vo + vn]),
                        start=(kc == 0),
                        stop=(kc == CT - 1),
                    )
            osb = work.tile([128, C], F32, name=f"osb{nt}", tag="sq")
            nc.vector.tensor_copy(out=osb, in_=ps)
            nc.sync.dma_start(out=out_ext[128 * nt:128 * (nt + 1), :], in_=osb)

    if compile_module:
        nc.compile()
    return nc


def prep_inputs(x, qkv_weight, proj_weight):
    """CPU-side input preparation: shard, transpose, fold LN centering."""
    x = np.asarray(x, dtype=np.float32)
    qkv_weight = np.asarray(qkv_weight, dtype=np.float32)
    proj_weight = np.asarray(proj_weight, dtype=np.float32)

    Wq = qkv_weight[0:C]
    Wk = qkv_weight[C:2 * C]
    Wv = qkv_weight[2 * C:3 * C]

    def center(W):
        Wc = W.reshape(H, D, C)
        Wc = Wc - Wc.mean(axis=1, keepdims=True)
        return Wc.reshape(C, C)

    wqcT = np.ascontiguousarray(center(Wq).T)
    wkcT = np.ascontiguousarray(center(Wk).T)
    wvT = np.ascontiguousarray(Wv.T)
    wpT = np.ascontiguousarray(proj_weight.T)

    seg = np.zeros((128, CT, H), np.float32)
    for r in range(CT):
        for j in range(2):
            seg[64 * j:64 * (j + 1), r, 2 * r + j] = 1.0
    ind = np.repeat(np.eye(H, dtype=np.float32), D, axis=1)  # [12, 768]
    id12 = np.eye(H, dtype=np.float32)

    in_maps = []
    for b in range(x.shape[0]):
        in_maps.append(dict(
            xT=np.ascontiguousarray(x[b].T),
            wqcT=wqcT, wkcT=wkcT, wvT=wvT, wpT=wpT,
            seg=seg, ind=ind, id12=id12,
        ))
    return in_maps


_CACHE = {}


def kernel(x, qkv_weight, proj_weight):
    if "nc" not in _CACHE:
        _CACHE["nc"] = build()
    nc = _CACHE["nc"]
    in_maps = prep_inputs(x, qkv_weight, proj_weight)
    from concourse.bass_utils import run_bass_kernel_spmd
    res = run_bass_kernel_spmd(nc, in_maps, core_ids=list(range(len(in_maps))))
    out = np.stack([res.results[i]["out"] for i in range(len(in_maps))], axis=0)
    return out.astype(np.float32)



# revision 5
# speedup vs baseline: 1.3209x; 1.3209x over previous
"""Trainium2 Bass kernel for nn_AttentionCompiled (dense transformer attention).

B=8, N=1024, C=768, H=12 heads, D=64. Per-head LayerNorm on q/k, softmax
attention, output projection.

Strategy: pure data parallelism — one batch element per NeuronCore (B=8 ==
n_cores), weights replicated, no collectives.

Math folding (validated vs reference to ~1e-6):
 - LayerNorm centering is folded into the QKV weights: Wq_h <- (I - J/64) Wq_h
   (and same for k). Row-sums of centered vectors vanish, so
   (q-mu_q).(k-mu_k) == qc.kc with both pre-centered by the weights.
 - 1/sigma_q (and the 1/sqrt(D) attention scale) is folded into q before the
   QK matmul: aq[n] = 1/sqrt(sum_d qc^2 + 64*eps) == (1/8)/sqrt(var+eps).
 - 1/sigma_k is applied as the per-partition `scale` operand of the Exp
   activation on S^T tiles (partition axis = m = key index).
 - Softmax has no max-subtraction: |scores| <= 8 by Cauchy-Schwarz on the
   normalized vectors, so exp is always in-range in fp32.
 - Softmax denominators come free from an appended ones-column on V:
   out^T = [v | 1]^T @ P^T gives rows 0..63 = unnormalized output, row 64 =
   sum_m P^T[m, n].
 - All activations live in transposed ([feature, token]) layouts so the
   attention chain needs zero on-chip transposes; x is pre-transposed on the
   CPU (free), and the final projection (lhsT = O^T slice) lands the output
   back in natural [token, feature] layout.

Perf structure (v1 rewrite):
 - Attention inner loop: per (pair, m-tile, head) one [128, 1024] PSUM score
   tile (both 512-wide query blocks) -> ONE Exp ACTIVATE with FD=1024
   (amortizes the ~350-cycle ACT instruction overhead) -> two PV matmuls.
   Score PSUM double-buffered (2 tiles x 2 banks); O^T accumulators
   [65, 1024] x2 = 4 banks. Total 8 PSUM banks.
 - Pair epilogue: O^T accumulators are copied PSUM->SBUF immediately (frees
   the banks for the next pair), then the softmax division happens off the
   critical path: reciprocal_approx_fast (single custom-DVE op, ~5x faster
   than the iterative-divide RECIPROCAL) + DMA broadcast + DVE multiply.
 - Stats sigma: Sqrt on ACT + reciprocal_approx_fast (replacing the
   6.5us-per-call vector.reciprocal on thin tiles).
"""

import sys
import numpy as np
from contextlib import ExitStack

if "/opt/trn_rl_repo" not in sys.path:
    sys.path.insert(0, "/opt/trn_rl_repo")

import concourse.bass as bass
import concourse.bacc as bacc
import concourse.tile as tile
from concourse import mybir

F32 = mybir.dt.float32
F32R = mybir.dt.float32r

N = 1024     # tokens per core
C = 768      # channels
H = 12       # heads
D = 64       # head dim
NT = N // 128   # 8 token tiles
CT = C // 128   # 6 channel tiles
NB = N // 512   # 2 free-dim blocks of 512
NP = H // 2     # 6 head pairs
EPS = 1e-5

USE_F32R = True  # fp32 matmuls run 4 cyc/row; float32r runs 1 cyc/row at N>=256


def build(use_f32r: bool = USE_F32R, compile_module: bool = True) -> bass.Bass:
    nc = bacc.Bacc()

    xT = nc.declare_dram_parameter("xT", [C, N], F32, isOutput=False)         # x[b].T
    wq = nc.declare_dram_parameter("wqcT", [C, C], F32, isOutput=False)       # centered Wq^T  [c, hd]
    wk = nc.declare_dram_parameter("wkcT", [C, C], F32, isOutput=False)       # centered Wk^T
    wv = nc.declare_dram_parameter("wvT", [C, C], F32, isOutput=False)        # Wv^T
    wp = nc.declare_dram_parameter("wpT", [C, C], F32, isOutput=False)        # proj_weight^T
    seg = nc.declare_dram_parameter("seg", [128, CT, H], F32, isOutput=False)  # per-c-tile head-segment-sum lhsT
    ind = nc.declare_dram_parameter("ind", [H, C], F32, isOutput=False)       # head->channel block indicator (bcast lhsT)
    id12 = nc.declare_dram_parameter("id12", [H, H], F32, isOutput=False)     # identity for PE transpose
    out_ext = nc.declare_dram_parameter("out", [N, C], F32, isOutput=True)

    MMD = F32R if use_f32r else F32

    def mm(ap):
        # bitcast for APs out of fp32-typed tensors; no-op if already f32r
        if use_f32r and ap.dtype != F32R:
            return ap.bitcast(F32R)
        return ap

    with tile.TileContext(nc) as tc, ExitStack() as ctx:
        # ---- pools ----
        persist = ctx.enter_context(tc.tile_pool(name="persist", bufs=1))
        # xT tiles and O^T tiles share slots (disjoint lifetimes), same for wv/wp
        xo_pool = ctx.enter_context(tc.tile_pool(name="xo", bufs=6))
        vp_pool = ctx.enter_context(tc.tile_pool(name="vp", bufs=6))
        work = ctx.enter_context(tc.tile_pool(name="work", bufs=2))
        ptp = ctx.enter_context(tc.tile_pool(name="ptp", bufs=3))
        epi = ctx.enter_context(tc.tile_pool(name="epi", bufs=1))
        rows = ctx.enter_context(tc.tile_pool(name="rows", bufs=1))
        dramp = ctx.enter_context(tc.tile_pool(name="dramp", bufs=2, space="DRAM"))
        # PSUM: psS = 2 x [128,1024] (4 banks) rotating for all single-shot
        # matmul outputs; psOT = 2 x [65,1024] (4 banks) O^T accumulators.
        psS = ctx.enter_context(tc.tile_pool(name="psS", bufs=2, space="PSUM"))
        psOT = ctx.enter_context(tc.tile_pool(name="psOT", bufs=1, space="PSUM"))

        # ---- load inputs to SBUF ----
        xt_sb = []
        for r in range(CT):
            t = xo_pool.tile([128, N], MMD, name=f"xt{r}", tag="xo")
            nc.sync.dma_start(out=t, in_=mm(xT[128 * r:128 * (r + 1), :]))
            xt_sb.append(t)

        def load_w(dram, nm, pool, tag, per_tile_tags=False):
            tiles = []
            for r in range(CT):
                t = pool.tile(
                    [128, C], MMD, name=f"{nm}{r}",
                    tag=(f"{tag}{r}" if per_tile_tags else tag),
                )
                nc.sync.dma_start(out=t, in_=mm(dram[128 * r:128 * (r + 1), :]))
                tiles.append(t)
            return tiles

        wq_sb = load_w(wq, "wq", persist, "wq", per_tile_tags=True)
        wk_sb = load_w(wk, "wk", persist, "wk", per_tile_tags=True)
        wv_sb = load_w(wv, "wv", vp_pool, "vp")

        seg_sb = rows.tile([128, CT, H], MMD, name="seg", tag="seg")
        nc.sync.dma_start(out=seg_sb, in_=mm(seg[:, :, :]))
        ind_sb = rows.tile([H, C], MMD, name="ind", tag="ind")
        nc.sync.dma_start(out=ind_sb, in_=mm(ind[:, :]))
        id12_sb = rows.tile([H, H], F32, name="id12", tag="id12")
        nc.sync.dma_start(out=id12_sb, in_=id12[:, :])

        epsq_sb = rows.tile([H, 1], F32, name="epsq", tag="epsq")
        nc.vector.memset(epsq_sb, float(D) * EPS)
        epsk_sb = rows.tile([H, 1], F32, name="epsk", tag="epsk")
        nc.vector.memset(epsk_sb, EPS)

        # ---- stage A: q^c.T and k^c.T  ([hd, n] layouts, 6 c-tiles each) ----
        qh_sb, kh_sb = [], []
        for nm, wt, outl in (("qh", wq_sb, qh_sb), ("kh", wk_sb, kh_sb)):
            for r in range(CT):
                ps = psS.tile([128, N], F32, name=f"ps_{nm}{r}", tag="s")
                for kc in range(CT):
                    for nb in range(NB):
                        nc.tensor.matmul(
                            ps[:, 512 * nb:512 * (nb + 1)],
                            lhsT=mm(wt[kc][:, 128 * r:128 * (r + 1)]),
                            rhs=mm(xt_sb[kc][:, 512 * nb:512 * (nb + 1)]),
                            start=(kc == 0),
                            stop=(kc == CT - 1),
                        )
                t = persist.tile([128, N], MMD, name=f"{nm}sb{r}", tag=f"{nm}{r}")
                nc.vector.tensor_copy(out=t, in_=ps)
                outl.append(t)

        # ---- stage A2: V in natural layout with ones column: [m, h, 65] ----
        v_sb = []
        for mt in range(NT):
            ps = psS.tile([128, C], F32, name=f"ps_v{mt}", tag="s")
            for kc in range(CT):
                for vo, vn in ((0, 512), (512, 256)):
                    nc.tensor.matmul(
                        ps[:, vo:vo + vn],
                        lhsT=mm(xt_sb[kc][:, 128 * mt:128 * (mt + 1)]),
                        rhs=mm(wv_sb[kc][:, vo:vo + vn]),
                        start=(kc == 0),
                        stop=(kc == CT - 1),
                    )
            t = persist.tile([128, H, D + 1], MMD, name=f"vsb{mt}", tag=f"v{mt}")
            # ones column for the softmax-denominator trick: fill with 1.0
            # first (contiguous memset), then overwrite cols 0..D-1 with v
            nc.vector.memset(t.bitcast(F32) if use_f32r else t, 1.0)
            nc.vector.tensor_copy(
                out=t[:, :, 0:D], in_=ps.rearrange("p (h d) -> p h d", h=H)
            )
            v_sb.append(t)

        # ---- stage B: per-head inverse std rows  aq, ak [12, 1024] ----
        stat_sb = {}
        for nm, src, eps_t, in (("aq", qh_sb, epsq_sb), ("ak", kh_sb, epsk_sb)):
            ps = psS.tile([H, N], F32, name=f"ps_{nm}", tag="s")
            for r in range(CT):
                sq = work.tile([128, N], MMD, name=f"sq_{nm}{r}", tag="sq")
                nc.vector.tensor_mul(out=sq, in0=src[r], in1=src[r])
                for nb in range(NB):
                    nc.tensor.matmul(
                        ps[:, 512 * nb:512 * (nb + 1)],
                        lhsT=mm(seg_sb[:, r, :]),
                        rhs=mm(sq[:, 512 * nb:512 * (nb + 1)]),
                        start=(r == 0),
                        stop=(r == CT - 1),
                    )
            t = persist.tile([H, N], F32, name=f"{nm}sb", tag=nm)
            # sigma-like row: sqrt(scale*sumsq + eps); then fast reciprocal
            nc.scalar.activation(
                out=t, in_=ps, func=mybir.ActivationFunctionType.Sqrt,
                bias=eps_t, scale=(1.0 if nm == "aq" else 1.0 / D),
            )
            nc.vector.reciprocal_approx_fast(out=t, in_=t)
            stat_sb[nm] = t
        aq_sb, ak_sb = stat_sb["aq"], stat_sb["ak"]
        if use_f32r:
            # f32r matmul operands must be pre-rounded: real CAST, not bitcast
            aq_r = persist.tile([H, N], F32R, name="aq_r", tag="aq_r")
            nc.vector.tensor_copy(out=aq_r, in_=aq_sb)
            aq_sb = aq_r

        # ---- stage C: scale q by broadcast(aq) (folds LN sigma + 1/sqrt(D)) ----
        for r in range(CT):
            ps = psS.tile([128, N], F32, name=f"ps_bq{r}", tag="s")
            for nb in range(NB):
                nc.tensor.matmul(
                    ps[:, 512 * nb:512 * (nb + 1)],
                    lhsT=mm(ind_sb[:, 128 * r:128 * (r + 1)]),
                    rhs=mm(aq_sb[:, 512 * nb:512 * (nb + 1)]),
                    start=True, stop=True,
                )
            nc.vector.tensor_mul(out=qh_sb[r], in0=qh_sb[r], in1=ps)

        # ---- stage D: ak columns per m-tile via PE transpose: akT[mt] [128, 12] ----
        akT_sb = []
        for mt in range(NT):
            ps = psS.tile([128, H], F32, name=f"ps_akT{mt}", tag="s")
            nc.tensor.transpose(
                out=ps, in_=ak_sb[:, 128 * mt:128 * (mt + 1)], identity=id12_sb
            )
            t = persist.tile([128, H], F32, name=f"akT{mt}", tag=f"akT{mt}")
            nc.vector.tensor_copy(out=t, in_=ps)
            akT_sb.append(t)

        # ---- stage E: attention ----
        ot_sb = []
        for r in range(CT):
            t = xo_pool.tile([128, N], MMD, name=f"ot{r}", tag="xo")
            ot_sb.append(t)

        for pr in range(NP):
            ot_ps = [
                psOT.tile([D + 1, N], F32, name=f"ps_ot{pr}_{j}", tag=f"ot{j}")
                for j in range(2)
            ]
            for mt in range(NT):
                for j in range(2):
                    h = 2 * pr + j
                    s_ps = psS.tile([128, N], F32, name=f"ps_s{h}_{mt}", tag="s")
                    for nb in range(NB):
                        nc.tensor.matmul(
                            s_ps[:, 512 * nb:512 * (nb + 1)],
                            lhsT=mm(kh_sb[pr][64 * j:64 * (j + 1), 128 * mt:128 * (mt + 1)]),
                            rhs=mm(qh_sb[pr][64 * j:64 * (j + 1), 512 * nb:512 * (nb + 1)]),
                            start=True, stop=True,
                        )
                    # one Exp over both 512-blocks (FD=1024) with per-key
                    # 1/sigma_k as the per-partition scale
                    pt = ptp.tile([128, N], MMD, name=f"pt{h}_{mt}", tag="pt")
                    nc.scalar.activation(
                        out=pt, in_=s_ps, func=mybir.ActivationFunctionType.Exp,
                        scale=akT_sb[mt][:, h:h + 1],
                    )
                    for nb in range(NB):
                        nc.tensor.matmul(
                            ot_ps[j][:, 512 * nb:512 * (nb + 1)],
                            lhsT=mm(v_sb[mt][:, h, :]),
                            rhs=mm(pt[:, 512 * nb:512 * (nb + 1)]),
                            start=(mt == 0),
                            stop=(mt == NT - 1),
                        )
            # epilogue: evacuate accumulators first (frees PSUM banks for the
            # next pair), then do the softmax division off the critical path.
            osb = []
            for j in range(2):
                t = epi.tile([D + 1, N], F32, name=f"osb{pr}_{j}", tag=f"osb{j}")
                nc.vector.tensor_copy(out=t, in_=ot_ps[j])
                osb.append(t)
            for j in range(2):
                h = 2 * pr + j
                # fast reciprocal of the denominator row. The custom DVE op
                # mishandles base_partition != 0, so run it over the whole
                # [65, N] tile (same cost — DVE time is free-dim-bound) and
                # use row 64 of the result.
                rr = epi.tile([D + 1, N], F32, name=f"rr{h}", tag="rr")
                nc.vector.reciprocal_approx_fast(out=rr, in_=osb[j])
                dn_dram = dramp.tile([1, N], F32, name=f"dnd{h}", tag="dnd")
                nc.gpsimd.dma_start(out=dn_dram, in_=rr[D:D + 1, :])
                rbc = epi.tile([D, N], F32, name=f"rbc{h}", tag=f"rbc{j}")
                bcast_ap = bass.AP(
                    tensor=dn_dram.tensor, offset=dn_dram.offset,
                    ap=[[0, D]] + [list(a) for a in dn_dram.ap[1:]],
                )
                nc.gpsimd.dma_start(out=rbc, in_=bcast_ap)
                if j == 0:
                    nc.vector.tensor_mul(
                        out=ot_sb[pr][0:D, :], in0=osb[j][0:D, :], in1=rbc
                    )
                else:
                    # rows 0..63 of rr are unused scratch — reuse as the temp
                    nc.vector.tensor_mul(out=rr[0:D, :], in0=osb[j][0:D, :], in1=rbc)
                    nc.gpsimd.dma_start(out=ot_sb[pr][D:2 * D, :], in_=mm(rr[0:D, :]))

        # ---- stage F: projection (lands output in natural [n, c] layout) ----
        wp_sb = load_w(wp, "wp", vp_pool, "vp")
        for nt in range(NT):
            ps = psS.tile([128, C], F32, name=f"ps_pj{nt}", tag="s")
            for kc in range(CT):
                for vo, vn in ((0, 512), (512, 256)):
                    nc.tensor.matmul(
                        ps[:, vo:vo + vn],
                        lhsT=mm(ot_sb[kc][:, 128 * nt:128 * (nt + 1)]),
                        rhs=mm(wp_sb[kc][:, vo:vo + vn]),
                        start=(kc == 0),
                        stop=(kc == CT - 1),
                    )
            osb = work.tile([128, C], F32, name=f"osb{nt}", tag="sq")
            nc.vector.tensor_copy(out=osb, in_=ps)
            nc.sync.dma_start(out=out_ext[128 * nt:128 * (nt + 1), :], in_=osb)

    if compile_module:
        nc.compile()
    return nc


def prep_inputs(x, qkv_weight, proj_weight):
    """CPU-side input preparation: shard, transpose, fold LN centering."""
    x = np.asarray(x, dtype=np.float32)
    qkv_weight = np.asarray(qkv_weight, dtype=np.float32)
    proj_weight = np.asarray(proj_weight, dtype=np.float32)

    Wq = qkv_weight[0:C]
    Wk = qkv_weight[C:2 * C]
    Wv = qkv_weight[2 * C:3 * C]

    def center(W):
        Wc = W.reshape(H, D, C)
        Wc = Wc - Wc.mean(axis=1, keepdims=True)
        return Wc.reshape(C, C)

    wqcT = np.ascontiguousarray(center(Wq).T)
    wkcT = np.ascontiguousarray(center(Wk).T)
    wvT = np.ascontiguousarray(Wv.T)
    wpT = np.ascontiguousarray(proj_weight.T)

    seg = np.zeros((128, CT, H), np.float32)
    for r in range(CT):
        for j in range(2):
            seg[64 * j:64 * (j + 1), r, 2 * r + j] = 1.0
    ind = np.repeat(np.eye(H, dtype=np.float32), D, axis=1)  # [12, 768]
    id12 = np.eye(H, dtype=np.float32)

    in_maps = []
    for b in range(x.shape[0]):
        in_maps.append(dict(
            xT=np.ascontiguousarray(x[b].T),
            wqcT=wqcT, wkcT=wkcT, wvT=wvT, wpT=wpT,
            seg=seg, ind=ind, id12=id12,
        ))
    return in_maps


_CACHE = {}


def kernel(x, qkv_weight, proj_weight):
    if "nc" not in _CACHE:
        _CACHE["nc"] = build()
    nc = _CACHE["nc"]
    in_maps = prep_inputs(x, qkv_weight, proj_weight)
    from concourse.bass_utils import run_bass_kernel_spmd
    res = run_bass_kernel_spmd(nc, in_maps, core_ids=list(range(len(in_maps))))
    out = np.stack([res.results[i]["out"] for i in range(len(in_maps))], axis=0)
    return out.astype(np.float32)


# revision 11
# speedup vs baseline: 1.4723x; 1.1146x over previous
"""Trainium2 Bass kernel for nn_AttentionCompiled (dense transformer attention).

B=8, N=1024, C=768, H=12 heads, D=64. Per-head LayerNorm on q/k, softmax
attention, output projection. Pure data parallelism: one batch element per
NeuronCore, weights replicated, no collectives.

Math folding:
 - LN centering folded into Wq/Wk (CPU-side).
 - 1/sigma_q (and 1/sqrt(D)) folded into q, 1/sigma_k folded into k, both via
   PE-broadcast matmul + DVE multiply, so Exp runs with scale=1 and any tiles
   can batch into one ACTIVATE.
 - 1/sigma via exp(-0.5*ln(sumsq*s + eps')) — ln and exp live in the SAME ACT
   table set (natural_log_exp_and_others), so per-pair stats interleave with
   attention exps with zero table reloads. q-stats live at partitions 0-1 and
   k-stats at partitions 32-33 of one [34, N] tile (matmul out base_partition
   must be 0/32/64); per-partition scale/bias APs select the right eps/scale.
 - Softmax denominators from an appended ones-column on V (row 64 of O^T).
 - |scores| <= 8 (Cauchy-Schwarz) so exp needs no max-subtraction.

Perf structure (v2):
 - Per-pair pipeline: pair p's attention stream has pair p+1's qkv-gen, stats,
   and k/q-prescale matmuls interleaved as PE filler between the ACT-paced
   exp ops. This keeps the PE instruction stream gapless, which keeps the HAM
   clock gate at K=8/8 (2.4 GHz) — v1 ran the whole attention phase at half
   clock because the PE idled ~1us every iteration.
 - Loop order pr -> nb -> mt. Score tile [128, 1024] holds both heads of the
   pair (row-group-concurrent QK matmuls), one Exp ACTIVATE with FD=1024.
   Score PSUM rotates 3 slots (6 banks); O^T accumulators [65, 512] x2 take
   the other 2 banks.
 - Epilogue per (pr, nb): evacuate accumulators to SBUF immediately (frees
   PSUM), reciprocal_approx_fast (custom DVE op; full-tile because it
   mishandles base_partition != 0), DMA broadcast via DRAM bounce, DVE mul.
"""

import sys
import numpy as np
from contextlib import ExitStack

if "/opt/trn_rl_repo" not in sys.path:
    sys.path.insert(0, "/opt/trn_rl_repo")

import concourse.bass as bass
import concourse.bacc as bacc
import concourse.tile as tile
from concourse import mybir

F32 = mybir.dt.float32
F32R = mybir.dt.float32r

N = 1024
C = 768
H = 12
D = 64
NT = N // 128
CT = C // 128
NB = N // 512
NP = H // 2
EPS = 1e-5

USE_F32R = True


def build(use_f32r: bool = USE_F32R, compile_module: bool = True) -> bass.Bass:
    nc = bacc.Bacc()

    xT = nc.declare_dram_parameter("xT", [C, N], F32, isOutput=False)
    wq = nc.declare_dram_parameter("wqcT", [C, C], F32, isOutput=False)
    wk = nc.declare_dram_parameter("wkcT", [C, C], F32, isOutput=False)
    wv = nc.declare_dram_parameter("wvT", [C, C], F32, isOutput=False)
    wp = nc.declare_dram_parameter("wpT", [C, C], F32, isOutput=False)
    seg = nc.declare_dram_parameter("seg", [128, CT, H], F32, isOutput=False)
    ind2d = nc.declare_dram_parameter("ind2", [34, 128], F32, isOutput=False)
    out_ext = nc.declare_dram_parameter("out", [N, C], F32, isOutput=True)

    MMD = F32R if use_f32r else F32

    def mm(ap):
        if use_f32r and ap.dtype != F32R:
            return ap.bitcast(F32R)
        return ap

    with tile.TileContext(nc) as tc, ExitStack() as ctx:
        persist = ctx.enter_context(tc.tile_pool(name="persist", bufs=1))
        xo_pool = ctx.enter_context(tc.tile_pool(name="xo", bufs=6))
        vp_pool = ctx.enter_context(tc.tile_pool(name="vp", bufs=6))
        work = ctx.enter_context(tc.tile_pool(name="work", bufs=2))
        ptp = ctx.enter_context(tc.tile_pool(name="ptp", bufs=3))
        epi = ctx.enter_context(tc.tile_pool(name="epi", bufs=1))
        stp = ctx.enter_context(tc.tile_pool(name="stp", bufs=1))
        qkp = ctx.enter_context(tc.tile_pool(name="qkp", bufs=1))
        rows = ctx.enter_context(tc.tile_pool(name="rows", bufs=1))
        dramp = ctx.enter_context(tc.tile_pool(name="dramp", bufs=2, space="DRAM"))
        psS = ctx.enter_context(tc.tile_pool(name="psS", bufs=3, space="PSUM"))
        psOT = ctx.enter_context(tc.tile_pool(name="psOT", bufs=1, space="PSUM"))

        # ---- loads ----
        xt_sb = []
        for r in range(CT):
            t = xo_pool.tile([128, N], MMD, name=f"xt{r}", tag="xo")
            nc.sync.dma_start(out=t, in_=mm(xT[128 * r:128 * (r + 1), :]))
            xt_sb.append(t)

        def load_w(dram, nm, pool, tag, per_tile_tags=False):
            tiles = []
            for r in range(CT):
                t = pool.tile(
                    [128, C], MMD, name=f"{nm}{r}",
                    tag=(f"{tag}{r}" if per_tile_tags else tag),
                )
                nc.sync.dma_start(out=t, in_=mm(dram[128 * r:128 * (r + 1), :]))
                tiles.append(t)
            return tiles

        wq_sb = load_w(wq, "wq", persist, "wq", per_tile_tags=True)
        wk_sb = load_w(wk, "wk", persist, "wk", per_tile_tags=True)
        wv_sb = load_w(wv, "wv", vp_pool, "vp")

        seg_sb = rows.tile([128, CT, H], MMD, name="seg", tag="seg")
        nc.sync.dma_start(out=seg_sb, in_=mm(seg[:, :, :]))

        # head->channel indicator rows for the q broadcast (partitions 0-1)
        # and the k broadcast (partitions 32-33), loaded from DRAM.
        ind2 = rows.tile([2, 128], MMD, name="ind2", tag="ind2")
        nc.sync.dma_start(out=ind2, in_=mm(ind2d[0:2, :]))

        epsq2 = rows.tile([2, 1], F32, name="epsq2", tag="epsq2")
        nc.vector.memset(epsq2, float(D) * EPS)
        epsk2 = rows.tile([2, 1], F32, name="epsk2", tag="epsk2")
        nc.vector.memset(epsk2, EPS)

        qh_sb = [None] * NP
        kh_sb = [None] * NP
        sig_r = [None] * NP

        def gen_qk(pr, which, part=None):
            """Emit matmuls producing qh[pr] or kh[pr]; part=0/1 emits half
            the contraction chunks (filler-sized), part=None emits all and
            the copy."""
            wt = wq_sb if which == "q" else wk_sb
            lst = qh_sb if which == "q" else kh_sb
            kcs = range(CT) if part is None else range(3 * part, 3 * part + 3)
            if part in (None, 0):
                ps = psS.tile([128, N], F32, name=f"ps_{which}{pr}", tag="s")
                gen_qk.ps[(pr, which)] = ps
            ps = gen_qk.ps[(pr, which)]
            for kc in kcs:
                for nb in range(NB):
                    nc.tensor.matmul(
                        ps[:, 512 * nb:512 * (nb + 1)],
                        lhsT=mm(wt[kc][:, 128 * pr:128 * (pr + 1)]),
                        rhs=mm(xt_sb[kc][:, 512 * nb:512 * (nb + 1)]),
                        start=(kc == 0),
                        stop=(kc == CT - 1),
                    )
            if part in (None, 1):
                t = qkp.tile([128, N], MMD, name=f"{which}h{pr}", tag=f"{which}h{pr}")
                nc.vector.tensor_copy(out=t, in_=ps)
                lst[pr] = t
        gen_qk.ps = {}

        def stats(pr):
            """sumsq -> 1/sigma for both heads of pair pr via
            exp(-0.5*ln(scale*sumsq + eps)) — ln/exp share one ACT table set
            with the attention Exp, so this interleaves with zero reloads."""
            sig_r[pr] = {}
            for which, src, eps_t, lsc in (
                ("q", qh_sb[pr], epsq2, 1.0), ("k", kh_sb[pr], epsk2, 1.0 / D)
            ):
                sq = work.tile([128, N], MMD, name=f"sq_{which}{pr}", tag="sq")
                nc.vector.tensor_mul(out=sq, in0=src, in1=src)
                ps2 = psS.tile([2, N], F32, name=f"ps_st{which}{pr}", tag="s")
                for nb in range(NB):
                    nc.tensor.matmul(
                        ps2[:, 512 * nb:512 * (nb + 1)],
                        lhsT=mm(seg_sb[:, pr, 2 * pr:2 * pr + 2]),
                        rhs=mm(sq[:, 512 * nb:512 * (nb + 1)]),
                        start=True, stop=True,
                    )
                ln_t = stp.tile([2, N], F32, name=f"ln{which}{pr}", tag="ln")
                nc.scalar.activation(
                    out=ln_t, in_=ps2, func=mybir.ActivationFunctionType.Ln,
                    bias=eps_t, scale=lsc,
                )
                inv = stp.tile([2, N], F32, name=f"inv{which}{pr}", tag="inv")
                nc.scalar.activation(
                    out=inv, in_=ln_t, func=mybir.ActivationFunctionType.Exp,
                    scale=-0.5,
                )
                sr = stp.tile([2, N], F32R if use_f32r else F32,
                              name=f"sigr{which}{pr}", tag=f"sigr{which}", bufs=2)
                nc.vector.tensor_copy(out=sr, in_=inv)  # real cast: f32r rounding
                sig_r[pr][which] = sr

        def prescale(pr, which):
            """Fold 1/sigma into qh or kh via PE row-broadcast + DVE mul."""
            tgt = qh_sb[pr] if which == "q" else kh_sb[pr]
            ps = psS.tile([128, N], F32, name=f"ps_b{which}{pr}", tag="s")
            for nb in range(NB):
                nc.tensor.matmul(
                    ps[:, 512 * nb:512 * (nb + 1)],
                    lhsT=mm(ind2[0:2, :]),
                    rhs=mm(sig_r[pr][which][:, 512 * nb:512 * (nb + 1)]),
                    start=True, stop=True,
                )
            nc.vector.tensor_mul(out=tgt, in0=tgt, in1=ps)

        # ---- prefix: pair 0 gen+stats+prescale, then V ----
        gen_qk(0, "q")
        gen_qk(0, "k")
        stats(0)
        prescale(0, "q")
        prescale(0, "k")

        v_sb = []
        for mt in range(NT):
            ps = psS.tile([128, C], F32, name=f"ps_v{mt}", tag="s")
            for kc in range(CT):
                for vo, vn in ((0, 512), (512, 256)):
                    nc.tensor.matmul(
                        ps[:, vo:vo + vn],
                        lhsT=mm(xt_sb[kc][:, 128 * mt:128 * (mt + 1)]),
                        rhs=mm(wv_sb[kc][:, vo:vo + vn]),
                        start=(kc == 0),
                        stop=(kc == CT - 1),
                    )
            t = persist.tile([128, H, D + 1], MMD, name=f"vsb{mt}", tag=f"v{mt}")
            nc.vector.memset(t.bitcast(F32) if use_f32r else t, 1.0)
            nc.vector.tensor_copy(
                out=t[:, :, 0:D], in_=ps.rearrange("p (h d) -> p h d", h=H)
            )
            v_sb.append(t)

        # ---- attention with interleaved next-pair generation ----
        ot_sb = [None] * CT

        def epilogue(pr, nb, ot_ps):
            if ot_sb[pr] is None:
                ot_sb[pr] = qkp.tile([128, N], MMD, name=f"ot{pr}", tag=f"qh{pr}")
            osb = []
            for j in range(2):
                t = epi.tile([D + 1, 512], F32, name=f"osb{pr}_{nb}_{j}", tag=f"osb{j}")
                nc.vector.tensor_copy(out=t, in_=ot_ps[j])
                osb.append(t)
            for j in range(2):
                h = 2 * pr + j
                rr = epi.tile([D + 1, 512], F32, name=f"rr{h}_{nb}", tag="rr")
                nc.vector.reciprocal_approx_fast(out=rr, in_=osb[j])
                dn_dram = dramp.tile([1, 512], F32, name=f"dnd{h}_{nb}", tag="dnd")
                nc.gpsimd.dma_start(out=dn_dram, in_=rr[D:D + 1, :])
                rbc = epi.tile([D, 512], F32, name=f"rbc{h}_{nb}", tag=f"rbc{j}")
                bcast_ap = bass.AP(
                    tensor=dn_dram.tensor, offset=dn_dram.offset,
                    ap=[[0, D]] + [list(a) for a in dn_dram.ap[1:]],
                )
                nc.gpsimd.dma_start(out=rbc, in_=bcast_ap)
                nsl = slice(512 * nb, 512 * (nb + 1))
                if j == 0:
                    nc.vector.tensor_mul(
                        out=ot_sb[pr][0:D, nsl], in0=osb[j][0:D, :], in1=rbc
                    )
                else:
                    nc.vector.tensor_mul(out=rr[0:D, :], in0=osb[j][0:D, :], in1=rbc)
                    nc.gpsimd.dma_start(out=ot_sb[pr][D:2 * D, nsl], in_=mm(rr[0:D, :]))

        for pr in range(NP):
            # filler chunks: next pair's gen/stats/prescale, spread over the
            # 16 (nb, mt) iterations of this pair's ACT-paced attention
            filler = []
            if pr + 1 < NP:
                nxt = pr + 1
                filler = [
                    lambda p=nxt: gen_qk(p, "q", 0),
                    lambda p=nxt: gen_qk(p, "q", 1),
                    lambda p=nxt: gen_qk(p, "k", 0),
                    lambda p=nxt: gen_qk(p, "k", 1),
                    lambda p=nxt: stats(p),
                    lambda p=nxt: prescale(p, "q"),
                    lambda p=nxt: prescale(p, "k"),
                ]
            fi = 0
            for nb in range(NB):
                ot_ps = [
                    psOT.tile([D + 1, 512], F32, name=f"ps_ot{pr}_{nb}_{j}", tag=f"ot{j}")
                    for j in range(2)
                ]
                for mt in range(NT):
                    s_ps = psS.tile([128, N], F32, name=f"ps_s{pr}_{nb}_{mt}", tag="s")
                    for j in range(2):
                        nc.tensor.matmul(
                            s_ps[:, 512 * j:512 * (j + 1)],
                            lhsT=mm(kh_sb[pr][64 * j:64 * (j + 1), 128 * mt:128 * (mt + 1)]),
                            rhs=mm(qh_sb[pr][64 * j:64 * (j + 1), 512 * nb:512 * (nb + 1)]),
                            start=True, stop=True,
                        )
                    pt = ptp.tile([128, N], MMD, name=f"pt{pr}_{nb}_{mt}", tag="pt")
                    nc.scalar.activation(
                        out=pt, in_=s_ps, func=mybir.ActivationFunctionType.Exp,
                    )
                    # PE filler while the Exp runs (keeps HAM warm and hides
                    # next-pair generation under this pair's attention)
                    if fi < len(filler) and (mt % 2 == 1):
                        filler[fi]()
                        fi += 1
                    for j in range(2):
                        nc.tensor.matmul(
                            ot_ps[j][:, :],
                            lhsT=mm(v_sb[mt][:, 2 * pr + j, :]),
                            rhs=mm(pt[:, 512 * j:512 * (j + 1)]),
                            start=(mt == 0),
                            stop=(mt == NT - 1),
                        )
                epilogue(pr, nb, ot_ps)
            while fi < len(filler):
                filler[fi]()
                fi += 1

        # ---- projection ----
        wp_sb = load_w(wp, "wp", vp_pool, "vp")
        for nt in range(NT):
            ps = psS.tile([128, C], F32, name=f"ps_pj{nt}", tag="s")
            for kc in range(CT):
                for vo, vn in ((0, 512), (512, 256)):
                    nc.tensor.matmul(
                        ps[:, vo:vo + vn],
                        lhsT=mm(ot_sb[kc][:, 128 * nt:128 * (nt + 1)]),
                        rhs=mm(wp_sb[kc][:, vo:vo + vn]),
                        start=(kc == 0),
                        stop=(kc == CT - 1),
                    )
            osb = work.tile([128, C], F32, name=f"osb{nt}", tag="sq")
            nc.vector.tensor_copy(out=osb, in_=ps)
            nc.sync.dma_start(out=out_ext[128 * nt:128 * (nt + 1), :], in_=osb)

    if compile_module:
        nc.compile()
    return nc


def prep_inputs(x, qkv_weight, proj_weight):
    x = np.asarray(x, dtype=np.float32)
    qkv_weight = np.asarray(qkv_weight, dtype=np.float32)
    proj_weight = np.asarray(proj_weight, dtype=np.float32)

    Wq = qkv_weight[0:C]
    Wk = qkv_weight[C:2 * C]
    Wv = qkv_weight[2 * C:3 * C]

    def center(W):
        Wc = W.reshape(H, D, C)
        Wc = Wc - Wc.mean(axis=1, keepdims=True)
        return Wc.reshape(C, C)

    wqcT = np.ascontiguousarray(center(Wq).T)
    wkcT = np.ascontiguousarray(center(Wk).T)
    wvT = np.ascontiguousarray(Wv.T)
    wpT = np.ascontiguousarray(proj_weight.T)

    seg = np.zeros((128, CT, H), np.float32)
    for r in range(CT):
        for j in range(2):
            seg[64 * j:64 * (j + 1), r, 2 * r + j] = 1.0
    ind2 = np.zeros((34, 128), np.float32)
    for base in (0, 32):
        ind2[base, 0:64] = 1.0
        ind2[base + 1, 64:128] = 1.0

    in_maps = []
    for b in range(x.shape[0]):
        in_maps.append(dict(
            xT=np.ascontiguousarray(x[b].T),
            wqcT=wqcT, wkcT=wkcT, wvT=wvT, wpT=wpT,
            seg=seg, ind2=ind2,
        ))
    return in_maps


_CACHE = {}


def kernel(x, qkv_weight, proj_weight):
    if "nc" not in _CACHE:
        _CACHE["nc"] = build()
    nc = _CACHE["nc"]
    in_maps = prep_inputs(x, qkv_weight, proj_weight)
    from concourse.bass_utils import run_bass_kernel_spmd
    res = run_bass_kernel_spmd(nc, in_maps, core_ids=list(range(len(in_maps))))
    out = np.stack([res.results[i]["out"] for i in range(len(in_maps))], axis=0)
    return out.astype(np.float32)


# revision 13
# speedup vs baseline: 1.6175x; 1.0986x over previous
"""Trainium2 Bass kernel for nn_AttentionCompiled (dense transformer attention).

B=8, N=1024, C=768, H=12 heads, D=64. Per-head LayerNorm on q/k, softmax
attention, output projection. Pure data parallelism: one batch element per
NeuronCore, weights replicated, no collectives.

Math folding:
 - LN centering folded into Wq/Wk (CPU-side).
 - 1/sigma_q (and 1/sqrt(D)) folded into q, 1/sigma_k folded into k, both via
   PE-broadcast matmul + DVE multiply, so Exp runs with scale=1 and any tiles
   can batch into one ACTIVATE.
 - 1/sigma via exp(-0.5*ln(sumsq*s + eps')) — ln and exp live in the SAME ACT
   table set (natural_log_exp_and_others), so per-pair stats interleave with
   attention exps with zero table reloads. q-stats live at partitions 0-1 and
   k-stats at partitions 32-33 of one [34, N] tile (matmul out base_partition
   must be 0/32/64); per-partition scale/bias APs select the right eps/scale.
 - Softmax denominators from an appended ones-column on V (row 64 of O^T).
 - |scores| <= 8 (Cauchy-Schwarz) so exp needs no max-subtraction.

Perf structure (v2):
 - Per-pair pipeline: pair p's attention stream has pair p+1's qkv-gen, stats,
   and k/q-prescale matmuls interleaved as PE filler between the ACT-paced
   exp ops. This keeps the PE instruction stream gapless, which keeps the HAM
   clock gate at K=8/8 (2.4 GHz) — v1 ran the whole attention phase at half
   clock because the PE idled ~1us every iteration.
 - Loop order pr -> nb -> mt. Score tile [128, 1024] holds both heads of the
   pair (row-group-concurrent QK matmuls), one Exp ACTIVATE with FD=1024.
   Score PSUM rotates 3 slots (6 banks); O^T accumulators [65, 512] x2 take
   the other 2 banks.
 - Epilogue per (pr, nb): evacuate accumulators to SBUF immediately (frees
   PSUM), reciprocal_approx_fast (custom DVE op; full-tile because it
   mishandles base_partition != 0), DMA broadcast via DRAM bounce, DVE mul.
"""

import sys
import numpy as np
from contextlib import ExitStack

if "/opt/trn_rl_repo" not in sys.path:
    sys.path.insert(0, "/opt/trn_rl_repo")

import concourse.bass as bass
import concourse.bacc as bacc
import concourse.tile as tile
from concourse import mybir

F32 = mybir.dt.float32
F32R = mybir.dt.float32r

N = 1024
C = 768
H = 12
D = 64
NT = N // 128
CT = C // 128
NB = N // 512
NP = H // 2
EPS = 1e-5

USE_F32R = True


def _filtered_act_tables(arch):
    """Drop the single-function exp/ln table sets so walrus resolves BOTH
    Ln and Exp to natural_log_exp_and_others — otherwise every Ln<->Exp
    transition in the interleaved stats/attention stream reloads the ACT
    table (~2.7us each, 24 reloads measured)."""
    import concourse.hw_specs as hw_specs
    tabs = dict(hw_specs.get_activation_tables(arch))
    # empty them (never match) rather than delete: act_func_set_id is the
    # INDEX into this ordered dict and must stay aligned with act_info.json
    for k in ("exp_and_others", "natural_log", "exp_and_friends"):
        tabs[k] = set()
    return tabs


def build(use_f32r: bool = USE_F32R, compile_module: bool = True) -> bass.Bass:
    bacc.get_activation_tables = _filtered_act_tables
    nc = bacc.Bacc()

    xT = nc.declare_dram_parameter("xT", [C, N], F32, isOutput=False)
    wq = nc.declare_dram_parameter("wqcT", [C, C], F32, isOutput=False)
    wk = nc.declare_dram_parameter("wkcT", [C, C], F32, isOutput=False)
    wv = nc.declare_dram_parameter("wvT", [C, C], F32, isOutput=False)
    wp = nc.declare_dram_parameter("wpT", [C, C], F32, isOutput=False)
    seg = nc.declare_dram_parameter("seg", [128, CT, H], F32, isOutput=False)
    ind2d = nc.declare_dram_parameter("ind2", [34, 128], F32, isOutput=False)
    out_ext = nc.declare_dram_parameter("out", [N, C], F32, isOutput=True)

    MMD = F32R if use_f32r else F32

    def mm(ap):
        if use_f32r and ap.dtype != F32R:
            return ap.bitcast(F32R)
        return ap

    with tile.TileContext(nc) as tc, ExitStack() as ctx:
        persist = ctx.enter_context(tc.tile_pool(name="persist", bufs=1))
        xo_pool = ctx.enter_context(tc.tile_pool(name="xo", bufs=6))
        vp_pool = ctx.enter_context(tc.tile_pool(name="vp", bufs=6))
        work = ctx.enter_context(tc.tile_pool(name="work", bufs=2))
        ptp = ctx.enter_context(tc.tile_pool(name="ptp", bufs=3))
        epi = ctx.enter_context(tc.tile_pool(name="epi", bufs=1))
        stp = ctx.enter_context(tc.tile_pool(name="stp", bufs=1))
        qkp = ctx.enter_context(tc.tile_pool(name="qkp", bufs=1))
        rows = ctx.enter_context(tc.tile_pool(name="rows", bufs=1))
        dramp = ctx.enter_context(tc.tile_pool(name="dramp", bufs=2, space="DRAM"))
        psS = ctx.enter_context(tc.tile_pool(name="psS", bufs=3, space="PSUM"))
        psOT = ctx.enter_context(tc.tile_pool(name="psOT", bufs=1, space="PSUM"))

        # ---- loads ----
        xt_sb = []
        for r in range(CT):
            t = xo_pool.tile([128, N], MMD, name=f"xt{r}", tag="xo")
            nc.sync.dma_start(out=t, in_=mm(xT[128 * r:128 * (r + 1), :]))
            xt_sb.append(t)

        def load_w(dram, nm, pool, tag, per_tile_tags=False):
            tiles = []
            for r in range(CT):
                t = pool.tile(
                    [128, C], MMD, name=f"{nm}{r}",
                    tag=(f"{tag}{r}" if per_tile_tags else tag),
                )
                nc.sync.dma_start(out=t, in_=mm(dram[128 * r:128 * (r + 1), :]))
                tiles.append(t)
            return tiles

        wq_sb = load_w(wq, "wq", persist, "wq", per_tile_tags=True)
        wk_sb = load_w(wk, "wk", persist, "wk", per_tile_tags=True)
        wv_sb = load_w(wv, "wv", vp_pool, "vp")

        seg_sb = rows.tile([128, CT, H], MMD, name="seg", tag="seg")
        nc.sync.dma_start(out=seg_sb, in_=mm(seg[:, :, :]))

        # head->channel indicator rows for the q broadcast (partitions 0-1)
        # and the k broadcast (partitions 32-33), loaded from DRAM.
        ind2 = rows.tile([2, 128], MMD, name="ind2", tag="ind2")
        nc.sync.dma_start(out=ind2, in_=mm(ind2d[0:2, :]))

        epsq2 = rows.tile([2, 1], F32, name="epsq2", tag="epsq2")
        nc.vector.memset(epsq2, float(D) * EPS)
        epsk2 = rows.tile([2, 1], F32, name="epsk2", tag="epsk2")
        nc.vector.memset(epsk2, EPS)

        qh_sb = [None] * NP
        kh_sb = [None] * NP
        sig_r = [None] * NP

        def gen_qk(pr, which, part=None):
            """Emit matmuls producing qh[pr] or kh[pr]; part=0/1 emits half
            the contraction chunks (filler-sized), part=None emits all and
            the copy."""
            wt = wq_sb if which == "q" else wk_sb
            lst = qh_sb if which == "q" else kh_sb
            kcs = range(CT) if part is None else range(3 * part, 3 * part + 3)
            if part in (None, 0):
                ps = psS.tile([128, N], F32, name=f"ps_{which}{pr}", tag="s")
                gen_qk.ps[(pr, which)] = ps
            ps = gen_qk.ps[(pr, which)]
            for kc in kcs:
                for nb in range(NB):
                    nc.tensor.matmul(
                        ps[:, 512 * nb:512 * (nb + 1)],
                        lhsT=mm(wt[kc][:, 128 * pr:128 * (pr + 1)]),
                        rhs=mm(xt_sb[kc][:, 512 * nb:512 * (nb + 1)]),
                        start=(kc == 0),
                        stop=(kc == CT - 1),
                    )
            if part in (None, 1):
                t = qkp.tile([128, N], MMD, name=f"{which}h{pr}", tag=f"{which}h{pr}")
                nc.vector.tensor_copy(out=t, in_=ps)
                lst[pr] = t
        gen_qk.ps = {}

        def stats(pr):
            """sumsq -> 1/sigma for both heads of pair pr via
            exp(-0.5*ln(scale*sumsq + eps)) — ln/exp share one ACT table set
            with the attention Exp, so this interleaves with zero reloads."""
            sig_r[pr] = {}
            for which, src, eps_t, lsc in (
                ("q", qh_sb[pr], epsq2, 1.0), ("k", kh_sb[pr], epsk2, 1.0 / D)
            ):
                sq = work.tile([128, N], MMD, name=f"sq_{which}{pr}", tag="sq")
                nc.vector.tensor_mul(out=sq, in0=src, in1=src)
                ps2 = psS.tile([2, N], F32, name=f"ps_st{which}{pr}", tag="s")
                for nb in range(NB):
                    nc.tensor.matmul(
                        ps2[:, 512 * nb:512 * (nb + 1)],
                        lhsT=mm(seg_sb[:, pr, 2 * pr:2 * pr + 2]),
                        rhs=mm(sq[:, 512 * nb:512 * (nb + 1)]),
                        start=True, stop=True,
                    )
                ln_t = stp.tile([2, N], F32, name=f"ln{which}{pr}", tag="ln")
                nc.scalar.activation(
                    out=ln_t, in_=ps2, func=mybir.ActivationFunctionType.Ln,
                    bias=eps_t, scale=lsc,
                )
                inv = stp.tile([2, N], F32, name=f"inv{which}{pr}", tag="inv")
                nc.scalar.activation(
                    out=inv, in_=ln_t, func=mybir.ActivationFunctionType.Exp,
                    scale=-0.5,
                )
                sr = stp.tile([2, N], F32R if use_f32r else F32,
                              name=f"sigr{which}{pr}", tag=f"sigr{which}", bufs=2)
                nc.vector.tensor_copy(out=sr, in_=inv)  # real cast: f32r rounding
                sig_r[pr][which] = sr

        def prescale(pr, which):
            """Fold 1/sigma into qh or kh via PE row-broadcast + DVE mul."""
            tgt = qh_sb[pr] if which == "q" else kh_sb[pr]
            ps = psS.tile([128, N], F32, name=f"ps_b{which}{pr}", tag="s")
            for nb in range(NB):
                nc.tensor.matmul(
                    ps[:, 512 * nb:512 * (nb + 1)],
                    lhsT=mm(ind2[0:2, :]),
                    rhs=mm(sig_r[pr][which][:, 512 * nb:512 * (nb + 1)]),
                    start=True, stop=True,
                )
            nc.vector.tensor_mul(out=tgt, in0=tgt, in1=ps)

        # ---- prefix: pair 0 gen+stats+prescale, then V ----
        gen_qk(0, "q")
        gen_qk(0, "k")
        stats(0)
        prescale(0, "q")
        prescale(0, "k")

        v_sb = []
        for mt in range(NT):
            ps = psS.tile([128, C], F32, name=f"ps_v{mt}", tag="s")
            for kc in range(CT):
                for vo, vn in ((0, 512), (512, 256)):
                    nc.tensor.matmul(
                        ps[:, vo:vo + vn],
                        lhsT=mm(xt_sb[kc][:, 128 * mt:128 * (mt + 1)]),
                        rhs=mm(wv_sb[kc][:, vo:vo + vn]),
                        start=(kc == 0),
                        stop=(kc == CT - 1),
                    )
            t = persist.tile([128, H, D + 1], MMD, name=f"vsb{mt}", tag=f"v{mt}")
            nc.vector.memset(t.bitcast(F32) if use_f32r else t, 1.0)
            nc.vector.tensor_copy(
                out=t[:, :, 0:D], in_=ps.rearrange("p (h d) -> p h d", h=H)
            )
            v_sb.append(t)

        # ---- attention with interleaved next-pair generation ----
        ot_sb = [None] * CT

        def epilogue(pr, nb, ot_ps):
            if ot_sb[pr] is None:
                ot_sb[pr] = qkp.tile([128, N], MMD, name=f"ot{pr}", tag=f"qh{pr}")
            osb = []
            for j in range(2):
                t = epi.tile([D + 1, 512], F32, name=f"osb{pr}_{nb}_{j}", tag=f"osb{j}")
                nc.vector.tensor_copy(out=t, in_=ot_ps[j])
                osb.append(t)
            for j in range(2):
                h = 2 * pr + j
                rr = epi.tile([D + 1, 512], F32, name=f"rr{h}_{nb}", tag="rr")
                nc.vector.reciprocal_approx_fast(out=rr, in_=osb[j])
                dn_dram = dramp.tile([1, 512], F32, name=f"dnd{h}_{nb}", tag="dnd")
                nc.gpsimd.dma_start(out=dn_dram, in_=rr[D:D + 1, :])
                rbc = epi.tile([D, 512], F32, name=f"rbc{h}_{nb}", tag=f"rbc{j}")
                bcast_ap = bass.AP(
                    tensor=dn_dram.tensor, offset=dn_dram.offset,
                    ap=[[0, D]] + [list(a) for a in dn_dram.ap[1:]],
                )
                nc.gpsimd.dma_start(out=rbc, in_=bcast_ap)
                nsl = slice(512 * nb, 512 * (nb + 1))
                if j == 0:
                    nc.vector.tensor_mul(
                        out=ot_sb[pr][0:D, nsl], in0=osb[j][0:D, :], in1=rbc
                    )
                else:
                    nc.vector.tensor_mul(out=rr[0:D, :], in0=osb[j][0:D, :], in1=rbc)
                    nc.gpsimd.dma_start(out=ot_sb[pr][D:2 * D, nsl], in_=mm(rr[0:D, :]))

        for pr in range(NP):
            # filler chunks: next pair's gen/stats/prescale, spread over the
            # 16 (nb, mt) iterations of this pair's ACT-paced attention
            filler = []
            if pr + 1 < NP:
                nxt = pr + 1
                filler = [
                    lambda p=nxt: gen_qk(p, "q", 0),
                    lambda p=nxt: gen_qk(p, "q", 1),
                    lambda p=nxt: gen_qk(p, "k", 0),
                    lambda p=nxt: gen_qk(p, "k", 1),
                    lambda p=nxt: stats(p),
                    lambda p=nxt: prescale(p, "q"),
                    lambda p=nxt: prescale(p, "k"),
                ]
            fi = 0
            for nb in range(NB):
                ot_ps = [
                    psOT.tile([D + 1, 512], F32, name=f"ps_ot{pr}_{nb}_{j}", tag=f"ot{j}")
                    for j in range(2)
                ]
                for mt in range(NT):
                    s_ps = psS.tile([128, N], F32, name=f"ps_s{pr}_{nb}_{mt}", tag="s")
                    for j in range(2):
                        nc.tensor.matmul(
                            s_ps[:, 512 * j:512 * (j + 1)],
                            lhsT=mm(kh_sb[pr][64 * j:64 * (j + 1), 128 * mt:128 * (mt + 1)]),
                            rhs=mm(qh_sb[pr][64 * j:64 * (j + 1), 512 * nb:512 * (nb + 1)]),
                            start=True, stop=True,
                        )
                    pt = ptp.tile([128, N], MMD, name=f"pt{pr}_{nb}_{mt}", tag="pt")
                    nc.scalar.activation(
                        out=pt, in_=s_ps, func=mybir.ActivationFunctionType.Exp,
                    )
                    # PE filler while the Exp runs (keeps HAM warm and hides
                    # next-pair generation under this pair's attention)
                    if fi < len(filler) and (mt % 2 == 1):
                        filler[fi]()
                        fi += 1
                    for j in range(2):
                        nc.tensor.matmul(
                            ot_ps[j][:, :],
                            lhsT=mm(v_sb[mt][:, 2 * pr + j, :]),
                            rhs=mm(pt[:, 512 * j:512 * (j + 1)]),
                            start=(mt == 0),
                            stop=(mt == NT - 1),
                        )
                epilogue(pr, nb, ot_ps)
            while fi < len(filler):
                filler[fi]()
                fi += 1

        # ---- projection ----
        wp_sb = load_w(wp, "wp", vp_pool, "vp")
        for nt in range(NT):
            ps = psS.tile([128, C], F32, name=f"ps_pj{nt}", tag="s")
            for kc in range(CT):
                for vo, vn in ((0, 512), (512, 256)):
                    nc.tensor.matmul(
                        ps[:, vo:vo + vn],
                        lhsT=mm(ot_sb[kc][:, 128 * nt:128 * (nt + 1)]),
                        rhs=mm(wp_sb[kc][:, vo:vo + vn]),
                        start=(kc == 0),
                        stop=(kc == CT - 1),
                    )
            osb = work.tile([128, C], F32, name=f"osb{nt}", tag="sq")
            nc.vector.tensor_copy(out=osb, in_=ps)
            nc.sync.dma_start(out=out_ext[128 * nt:128 * (nt + 1), :], in_=osb)

    if compile_module:
        nc.compile()
    return nc


def prep_inputs(x, qkv_weight, proj_weight):
    x = np.asarray(x, dtype=np.float32)
    qkv_weight = np.asarray(qkv_weight, dtype=np.float32)
    proj_weight = np.asarray(proj_weight, dtype=np.float32)

    Wq = qkv_weight[0:C]
    Wk = qkv_weight[C:2 * C]
    Wv = qkv_weight[2 * C:3 * C]

    def center(W):
        Wc = W.reshape(H, D, C)
        Wc = Wc - Wc.mean(axis=1, keepdims=True)
        return Wc.reshape(C, C)

    wqcT = np.ascontiguousarray(center(Wq).T)
    wkcT = np.ascontiguousarray(center(Wk).T)
    wvT = np.ascontiguousarray(Wv.T)
    wpT = np.ascontiguousarray(proj_weight.T)

    seg = np.zeros((128, CT, H), np.float32)
    for r in range(CT):
        for j in range(2):
            seg[64 * j:64 * (j + 1), r, 2 * r + j] = 1.0
    ind2 = np.zeros((34, 128), np.float32)
    for base in (0, 32):
        ind2[base, 0:64] = 1.0
        ind2[base + 1, 64:128] = 1.0

    in_maps = []
    for b in range(x.shape[0]):
        in_maps.append(dict(
            xT=np.ascontiguousarray(x[b].T),
            wqcT=wqcT, wkcT=wkcT, wvT=wvT, wpT=wpT,
            seg=seg, ind2=ind2,
        ))
    return in_maps


_CACHE = {}


def kernel(x, qkv_weight, proj_weight):
    if "nc" not in _CACHE:
        _CACHE["nc"] = build()
    nc = _CACHE["nc"]
    in_maps = prep_inputs(x, qkv_weight, proj_weight)
    from concourse.bass_utils import run_bass_kernel_spmd
    res = run_bass_kernel_spmd(nc, in_maps, core_ids=list(range(len(in_maps))))
    out = np.stack([res.results[i]["out"] for i in range(len(in_maps))], axis=0)
    return out.astype(np.float32)
